# revision 2
# baseline (speedup 1.0000x reference)
"""Trainium2 Bass kernel for nn_Defaultloss_49873160241482 (focal-BCE + smooth-L1 detection loss).

Self-contained: kernel(**inputs) takes full unsharded inputs, shards the batch
dim across 8 NeuronCores (2 batches/core), pads anchors A->A_PAD with neutral
values (assign=-1 so pads contribute nothing), runs the Bass program via
run_bass_kernel_spmd, and combines per-core partial sums on the host.

Math (per batch):
  cls: all 21 channels contribute L0(p) = 0.75*p^2*(-ln(1-p)) masked by
       (assign>=0); positive anchors get corrections Delta(p) = L1(p)-L0(p)
       at the obj channel and the assigned class channel, where
       L1(p) = 0.25*(1-p)^2*(-ln p) = L0(1-p)/3.
  box: smooth-L1 between anchor-relative targets and dt[0:4], via
       sl1(|d|) = (0.5/beta)*min(d^2,beta^2) + relu(d-beta) + relu(-d-beta).
  Per-anchor gt rows come from a gpsimd ap_gather with a field-per-partition
  table (row 16c+r of the table tile holds field r for Q7 core c).
Device returns per-batch partial sums; host divides by n_pos and B.
"""

import numpy as np

import concourse.bass as bass
import concourse.bacc as bacc
import concourse.mybir as mybir
import concourse.tile as tile

F32 = mybir.dt.float32
I32 = mybir.dt.int32
I16 = mybir.dt.int16
AF = mybir.ActivationFunctionType
OP = mybir.AluOpType

B, A, C, G = 16, 120000, 20, 64
ALPHA, GAMMA, BETA = 0.25, 2.0, 1.0 / 9.0
NCORES = 8
BPC = B // NCORES  # batches per core

NCH = 21   # 1 obj + 20 class channels (dt rows 4..24)
NQ = 2     # ap_gather call splits (each covers 8 of the 16 partition rows/core)


class Cfg:
    def __init__(self, A_pad):
        self.A = A_pad
        self.P_B = 128
        self.F_B = A_pad // 128          # box layout [128, F_B]
        assert A_pad % (128 * 16) == 0   # per-core 16*F_B indices, 16-wrapped
        self.CH_A = A_pad // 6           # cls: 6 chunks x 21 channels
        self.F_C = 512
        assert self.CH_A % self.F_C == 0 and A_pad % 6 == 0
        self.T_C = self.CH_A // self.F_C
        self.NG = 2 if self.T_C % 2 == 0 and self.T_C >= 4 else 1
        self.G_T = self.T_C // self.NG
        assert self.G_T <= 21
        self.PACK_P = 6 * self.G_T
        self.NCOL = 24
        assert self.F_B % 4 == 0


A_PAD = 122880  # = 128*960 = 6*40*512; pads get assign=-1 -> contribute zero

# output column indices (per batch)
COL_CLS0 = 0      # + group g
COL_NPOS0 = 2
COL_PC0 = 4
COL_OBJ0 = 6
COL_RELUP = 8     # + box channel j (4)
COL_RELUN = 12    # + j
COL_MSQ = 16      # + j


def _register_const_aps(nc, values):
    for value in values:
        t = nc.alloc_sbuf_tensor(f"const-f32-{value}", [128, 1], F32)
        nc.gpsimd.memset(t.ap(), value)
        nc.const_aps.aps[(F32, value)] = t.ap()
    nc.all_engine_barrier()


def build_program(cfg):
    nc = bacc.Bacc("TRN2", target_bir_lowering=False, debug=False)
    _register_const_aps(nc, [-BETA])

    Ap = cfg.A
    dtc = nc.dram_tensor("dtc", [BPC, 5 + C, Ap], F32, kind="ExternalInput")
    gtc = nc.dram_tensor("gtc", [BPC, G, 5], F32, kind="ExternalInput")
    anc = nc.dram_tensor("anc", [Ap, 4], F32, kind="ExternalInput")
    asg = nc.dram_tensor("asg", [BPC, Ap], I32, kind="ExternalInput")
    asg_il = nc.dram_tensor("asg_il", [BPC, Ap], I32, kind="ExternalInput")
    ones_dn = nc.dram_tensor("ones_dn", [126, 366], F32, kind="ExternalInput")
    iota_cls = nc.dram_tensor("iota_cls", [126, 1], F32, kind="ExternalInput")
    ones_col = nc.dram_tensor("ones_col", [128, 1], F32, kind="ExternalInput")
    ones_up = nc.dram_tensor("ones_up", [6, 126], F32, kind="ExternalInput")
    tblD = [nc.dram_tensor(f"tblD{b}", [G, 5], F32) for b in range(BPC)]
    clsD = nc.dram_tensor("clsD", [BPC, Ap], F32)
    out = nc.dram_tensor("out", [BPC, cfg.NCOL], F32, kind="ExternalOutput")

    with tile.TileContext(nc) as tc:
        with (
            tc.tile_pool(name="const", bufs=1) as cpool,
            tc.tile_pool(name="anch", bufs=1) as apool,
            tc.tile_pool(name="big", bufs=1) as bigpool,
            tc.tile_pool(name="cls", bufs=2) as clspool,
            tc.tile_pool(name="small", bufs=1) as spool,
            tc.tile_pool(name="pipe", bufs=1) as ppool,
            tc.tile_pool(name="pk", bufs=1) as pkpool,
            tc.tile_pool(name="psum", bufs=1, space="PSUM") as pspool,
            tc.tile_pool(name="psum_rep", bufs=2, space="PSUM") as reppool,
        ):
            ks = {}
            k_dn = cpool.tile([126, 366], F32, tag="k_dn")
            nc.sync.dma_start(out=k_dn[:], in_=ones_dn[:, :])
            k_up = cpool.tile([6, 126], F32, tag="k_up")
            nc.sync.dma_start(out=k_up[:], in_=ones_up[:, :])
            k_iota = cpool.tile([126, 1], F32, tag="k_iota")
            nc.sync.dma_start(out=k_iota[:], in_=iota_cls[:, :])
            k_ones = cpool.tile([128, 1], F32, tag="k_ones")
            nc.sync.dma_start(out=k_ones[:], in_=ones_col[:, :])
            ks.update(k_dn=k_dn, k_up=k_up, k_iota=k_iota, k_ones=k_ones)

            # ---- anchor precompute (shared by both batches), box layout ----
            P_B, F_B = cfg.P_B, cfg.F_B
            an = {}
            for nm in ("iaw", "iah", "axw", "ayh", "lnaw", "lnah"):
                an[nm] = apool.tile([P_B, F_B], F32, tag=nm, name=nm)

            for lo, hi, ia, lna, acw in (
                (0, 2, "iaw", "lnaw", "axw"), (1, 3, "iah", "lnah", "ayh")):
                sA = apool.tile([P_B, F_B], F32, tag="sA", name=f"sA_{lo}")
                sB = apool.tile([P_B, F_B], F32, tag="sB", name=f"sB_{lo}")
                for col, dst in ((lo, sA), (hi, sB)):
                    v = anc[:, col].rearrange("(p f) -> p f", p=P_B)
                    nc.sync.dma_start(out=dst[:64, :], in_=v[:64])
                    nc.sync.dma_start(out=dst[64:, :], in_=v[64:])
                nc.vector.tensor_tensor(sB[:], sB[:], sA[:], OP.subtract)  # a_w
                nc.vector.reciprocal(an[ia][:], sB[:])
                nc.scalar.activation(an[lna][:], sB[:], AF.Ln)
                nc.vector.scalar_tensor_tensor(sA[:], sB[:], 0.5, sA[:],
                                               OP.mult, OP.add)            # a_c
                nc.vector.tensor_tensor(an[acw][:], sA[:], an[ia][:], OP.mult)

            for b in range(BPC):
                build_batch(nc, tc, cfg, b, dtc=dtc, gtc=gtc, asg=asg,
                            asg_il=asg_il, tblD=tblD[b], clsD=clsD,
                            out=out, ks=ks, an=an, bigpool=bigpool,
                            clspool=clspool, spool=spool, ppool=ppool,
                            pkpool=pkpool, pspool=pspool, reppool=reppool)

    nc.compile()
    return nc


def build_batch(nc, tc, cfg, b, *, dtc, gtc, asg, asg_il, tblD, clsD, out, ks, an,
                bigpool, clspool, spool, ppool, pkpool, pspool, reppool):
    P_B, F_B = cfg.P_B, cfg.F_B
    F_C, T_C, G_T, NG = cfg.F_C, cfg.T_C, cfg.G_T, cfg.NG
    PACK_P = cfg.PACK_P
    k_dn, k_up, k_iota, k_ones = ks["k_dn"], ks["k_up"], ks["k_iota"], ks["k_ones"]

    # ---------- gt table prep: [Gx, Gy, ln w, ln h, cls-1] -> tblD ----------
    gt_s = spool.tile([G, 5], F32, tag="gt_s")
    nc.sync.dma_start(out=gt_s[:], in_=gtc[b, :, :])
    tblS = spool.tile([G, 5], F32, tag="tblS")
    g0, g1, g2, g3, g4 = (gt_s[:, j:j + 1] for j in range(5))
    nc.vector.scalar_tensor_tensor(tblS[:, 0:1], g2, 0.5, g0, OP.mult, OP.add)
    nc.vector.scalar_tensor_tensor(tblS[:, 1:2], g3, 0.5, g1, OP.mult, OP.add)
    nc.scalar.activation(tblS[:, 2:3], g2, AF.Ln)
    nc.scalar.activation(tblS[:, 3:4], g3, AF.Ln)
    nc.vector.tensor_scalar(tblS[:, 4:5], g4, 1.0, None, OP.subtract)
    nc.sync.dma_start(out=tblD[:, :], in_=tblS[:])
    # field-per-partition table tile: row 16c+r = field r (r<5), else 0
    tblT = spool.tile([128, G], F32, tag="tblT")
    nc.vector.memset(tblT[:], 0.0)
    for c in range(8):
        nc.sync.dma_start(out=tblT[16 * c:16 * c + 5, :],
                          in_=tblD[:, :].rearrange("g f -> f g"))

    # ---------- assign loads / gather indices ----------
    asg_box_i = spool.tile([P_B, F_B], I32, tag="asg_box_i")
    nc.sync.dma_start(out=asg_box_i[:], in_=asg[b, :].rearrange("(p f) -> p f", p=P_B))
    asg_box = spool.tile([P_B, F_B], F32, tag="asg_box")
    nc.vector.tensor_copy(asg_box[:], asg_box_i[:])
    il_i = spool.tile([P_B, F_B], I32, tag="il_i")
    nc.sync.dma_start(out=il_i[:], in_=asg_il[b, :].rearrange("(p f) -> p f", p=P_B))
    gidx32 = spool.tile([P_B, F_B], I32, tag="gidx32")
    nc.vector.tensor_scalar(gidx32[:], il_i[:], 1, 0, OP.subtract, OP.max)
    idx16 = spool.tile([P_B, F_B], I16, tag="idx16")
    nc.vector.tensor_copy(idx16[:], gidx32[:])

    # ---------- ap_gather: NQ calls, each covers 16/NQ w-rows per core ----------
    fld = {}
    for r, nm in enumerate(("Gx", "Gy", "LNw", "LNh", "CLS")):
        fld[nm] = bigpool.tile([P_B, F_B], F32, tag=f"fld_{nm}", name=f"fld_{nm}_{b}")
    nidx_q = 16 * F_B // NQ
    WQ = 16 // NQ
    for q in range(NQ):
        goutq = bigpool.tile([128, nidx_q], F32, tag="gout", name=f"gout_{b}_{q}")
        nc.gpsimd.ap_gather(
            out_ap=goutq[:].unsqueeze(-1),
            in_ap=tblT[:].unsqueeze(-1),
            idxs_ap=idx16[:, q * (nidx_q // 16):(q + 1) * (nidx_q // 16)],
            channels=128, num_elems=G, d=1, num_idxs=nidx_q,
        )
        for c in range(8):
            for r, nm in enumerate(("Gx", "Gy", "LNw", "LNh", "CLS")):
                nc.sync.dma_start(
                    out=fld[nm][16 * c + WQ * q:16 * c + WQ * (q + 1), :],
                    in_=goutq[16 * c + r:16 * c + r + 1, :].rearrange(
                        "p (w s) -> p w s", w=WQ),
                )

    # ---------- box path ----------
    strip = spool.tile([128, cfg.NCOL], F32, tag="strip")
    nc.vector.memset(strip[:], 0.0)
    dbig = bigpool.tile([P_B, 4, F_B], F32, tag="dbig")
    for j, (Gc, ia, acw) in enumerate(
        [(fld["Gx"], an["iaw"], an["axw"]), (fld["Gy"], an["iah"], an["ayh"])]
    ):
        t1 = ppool.tile([P_B, F_B], F32, tag="bx_t1")
        nc.vector.tensor_tensor(t1[:], Gc[:], ia[:], OP.mult)
        dl = ppool.tile([P_B, F_B], F32, tag="bx_dl")
        nc.sync.dma_start(out=dl[:], in_=dtc[b, j, :].rearrange("(p f) -> p f", p=P_B))
        cxy = ppool.tile([P_B, F_B], F32, tag="bx_c")
        nc.vector.tensor_tensor(cxy[:], acw[:], dl[:], OP.add)
        nc.vector.tensor_tensor(dbig[:, j, :], t1[:], cxy[:], OP.subtract)
    for j, (Lc, lna) in enumerate([(fld["LNw"], an["lnaw"]), (fld["LNh"], an["lnah"])]):
        dl = ppool.tile([P_B, F_B], F32, tag="bx_dl")
        nc.sync.dma_start(out=dl[:],
                          in_=dtc[b, 2 + j, :].rearrange("(p f) -> p f", p=P_B))
        cwh = ppool.tile([P_B, F_B], F32, tag="bx_c")
        nc.vector.tensor_tensor(cwh[:], lna[:], dl[:], OP.add)
        nc.vector.tensor_tensor(dbig[:, 2 + j, :], Lc[:], cwh[:], OP.subtract)
    # poison non-positives in place, then per-channel SL1 pieces (fused accums)
    for j in range(4):
        dj = dbig[:, j, :]
        nc.vector.scalar_tensor_tensor(dj, asg_box[:], 1.0, dj, OP.is_ge, OP.mult)
        junk = ppool.tile([P_B, F_B], F32, tag="junk")
        nc.scalar.activation(junk[:], dj, AF.Relu, bias=-BETA, scale=1.0,
                             accum_out=strip[:P_B, COL_RELUP + j:COL_RELUP + j + 1])
        junk2 = ppool.tile([P_B, F_B], F32, tag="junk")
        nc.scalar.activation(junk2[:], dj, AF.Relu, bias=-BETA, scale=-1.0,
                             accum_out=strip[:P_B, COL_RELUN + j:COL_RELUN + j + 1])
        sqd = ppool.tile([P_B, F_B], F32, tag="mj")
        nc.scalar.activation(sqd[:], dj, AF.Square)
        junk3 = ppool.tile([P_B, F_B], F32, tag="junk")
        nc.vector.tensor_scalar(junk3[:], sqd[:], BETA * BETA, None, OP.min, OP.add,
                                accum_out=strip[:P_B, COL_MSQ + j:COL_MSQ + j + 1])

    # ---------- poisoned class -> DRAM -> per-tile [6, F_C] chunks ----------
    clsP = spool.tile([P_B, F_B], F32, tag="clsP")
    nc.vector.tensor_scalar(clsP[:], fld["CLS"][:], 7.0, None, OP.add)
    nc.vector.scalar_tensor_tensor(clsP[:], asg_box[:], 1.0, clsP[:], OP.is_ge, OP.mult)
    nc.vector.tensor_scalar(clsP[:], clsP[:], 7.0, None, OP.subtract)
    nc.sync.dma_start(out=clsD[b, :].rearrange("(p f) -> p f", p=P_B), in_=clsP[:])
    cls_view = clsD[b, :].rearrange("(k t f) -> t k f", k=6, f=F_C)

    # ---------- cls main loop ----------
    ps_S0 = [pspool.tile([126, F_C], F32, tag=f"ps_S0_{g}", name=f"ps_S0_{g}_{b}")
             for g in range(NG)]
    ps_pc = [pspool.tile([126, F_C], F32, tag=f"ps_pc_{g}", name=f"ps_pc_{g}_{b}")
             for g in range(NG)]
    dview = dtc[b, 4:, :].rearrange("c (k t f) -> t k c f", k=6, f=F_C)
    for t in range(T_C):
        g, tl = t // G_T, t % G_T
        p_t = clspool.tile([126, F_C], F32, tag="p_t")
        nc.sync.dma_start(out=p_t[:], in_=dview[t])
        ln1m = clspool.tile([126, F_C], F32, tag="ln1m")
        nc.scalar.activation(ln1m[:], p_t[:], AF.Ln, bias=1.0, scale=-1.0)
        sq = clspool.tile([126, F_C], F32, tag="sq")
        nc.scalar.activation(sq[:], p_t[:], AF.Square)
        prod = clspool.tile([126, F_C], F32, tag="prod")
        nc.vector.scalar_tensor_tensor(prod[:], sq[:], -0.75, ln1m[:], OP.mult, OP.mult)
        nc.tensor.matmul(ps_S0[g][:, :], lhsT=k_dn[:, 240 - 6 * tl:366 - 6 * tl],
                         rhs=prod[:], start=(tl == 0), stop=(tl == G_T - 1))
        cls_t = clspool.tile([6, F_C], F32, tag="cls_t")
        nc.sync.dma_start(out=cls_t[:], in_=cls_view[t])
        rep = reppool.tile([126, F_C], F32, tag="rep")
        nc.tensor.matmul(rep[:], lhsT=k_up[:], rhs=cls_t[:], start=True, stop=True)
        z = clspool.tile([126, F_C], F32, tag="z")
        nc.vector.scalar_tensor_tensor(z[:], rep[:], k_iota[:, 0:1], p_t[:],
                                       OP.is_equal, OP.mult)
        nc.tensor.matmul(ps_pc[g][:, :], lhsT=k_dn[:, 240 - 6 * tl:366 - 6 * tl],
                         rhs=z[:], start=(tl == 0), stop=(tl == G_T - 1))

    # ---------- per-group packed reductions ----------
    aview = asg[b, :].rearrange("(k t f) -> t k f", k=6, f=F_C)
    d4view = dtc[b, 4, :].rearrange("(k t f) -> t k f", k=6, f=F_C)
    for g in range(NG):
        asg_pk_i = pkpool.tile([PACK_P, F_C], I32, tag="asg_pk_i")
        nc.sync.dma_start(out=asg_pk_i[:], in_=aview[g * G_T:(g + 1) * G_T])
        asg_pk = pkpool.tile([PACK_P, F_C], F32, tag="asg_pk")
        nc.vector.tensor_copy(asg_pk[:], asg_pk_i[:])
        jk = pkpool.tile([PACK_P, F_C], F32, tag="jk")
        nc.vector.scalar_tensor_tensor(
            jk[:], asg_pk[:], 0.0, ps_S0[g][:PACK_P, :], OP.is_ge, OP.mult,
            accum_out=strip[:PACK_P, COL_CLS0 + g:COL_CLS0 + g + 1])
        jk2 = pkpool.tile([PACK_P, F_C], F32, tag="jk")
        nc.vector.tensor_scalar(
            jk2[:], asg_pk[:], 1.0, None, OP.is_ge, OP.add,
            accum_out=strip[:PACK_P, COL_NPOS0 + g:COL_NPOS0 + g + 1])
        pc = pkpool.tile([PACK_P, F_C], F32, tag="pc")
        nc.vector.tensor_scalar(pc[:], asg_pk[:], 1.0, 0.5, OP.is_lt, OP.mult)
        nc.vector.tensor_tensor(pc[:], ps_pc[g][:PACK_P, :], pc[:], OP.add)
        delta_masked_sum(nc, pkpool, pc, asg_pk,
                         strip[:PACK_P, COL_PC0 + g:COL_PC0 + g + 1], PACK_P, F_C)
        p0 = pkpool.tile([PACK_P, F_C], F32, tag="p0")
        nc.sync.dma_start(out=p0[:], in_=d4view[g * G_T:(g + 1) * G_T])
        delta_masked_sum(nc, pkpool, p0, asg_pk,
                         strip[:PACK_P, COL_OBJ0 + g:COL_OBJ0 + g + 1], PACK_P, F_C)

    # ---------- finalize ----------
    ps_fin = reppool.tile([1, cfg.NCOL], F32, tag="ps_fin")
    nc.tensor.matmul(ps_fin[:], lhsT=k_ones[:], rhs=strip[:], start=True, stop=True)
    fin = spool.tile([1, cfg.NCOL], F32, tag="fin")
    nc.vector.tensor_copy(fin[:], ps_fin[:])
    nc.sync.dma_start(out=out[b, :].unsqueeze(0), in_=fin[:])


def delta_masked_sum(nc, pool, pv, asg_pk, acc_col, P, F):
    """acc_col = sum_free (asg>=1) * Delta(pv);
    Delta(p) = -0.25*(1-p)^2*ln(p) + 0.75*p^2*ln(1-p)."""
    lnp = pool.tile([P, F], F32, tag="dc_lnp")
    nc.scalar.activation(lnp[:], pv[:], AF.Ln)
    ln1mp = pool.tile([P, F], F32, tag="dc_ln1mp")
    nc.scalar.activation(ln1mp[:], pv[:], AF.Ln, bias=1.0, scale=-1.0)
    sqp = pool.tile([P, F], F32, tag="dc_sqp")
    nc.scalar.activation(sqp[:], pv[:], AF.Square)
    sq1mp = pool.tile([P, F], F32, tag="dc_sq1mp")
    nc.scalar.activation(sq1mp[:], pv[:], AF.Square, bias=1.0, scale=-1.0)
    nc.vector.scalar_tensor_tensor(sq1mp[:], sq1mp[:], -0.25, lnp[:], OP.mult, OP.mult)
    nc.vector.scalar_tensor_tensor(sqp[:], sqp[:], 0.75, ln1mp[:], OP.mult, OP.mult)
    nc.vector.tensor_tensor(sqp[:], sq1mp[:], sqp[:], OP.add)
    jk = pool.tile([P, F], F32, tag="dc_jk")
    nc.vector.scalar_tensor_tensor(jk[:], asg_pk[:], 1.0, sqp[:], OP.is_ge, OP.mult,
                                   accum_out=acc_col)


def make_consts():
    # sliding-window pack matrix: cls tile tl uses ones_dn[:, 240-6*tl : 366-6*tl],
    # which has a 1 at [p, 6*tl + p//21].
    ones_dn = np.zeros((126, 366), np.float32)
    for p in range(126):
        ones_dn[p, 240 + p // NCH] = 1.0
    ones_up = np.zeros((6, 126), np.float32)
    for m in range(126):
        ones_up[m // NCH, m] = 1.0
    iota_cls = (np.arange(126, dtype=np.float32) % NCH - 1.0).reshape(126, 1)
    ones_col = np.ones((128, 1), np.float32)
    return ones_dn, ones_up, iota_cls, ones_col


def interleave_assign(asg_pad):
    """asg_il[..., (16c+w)*F_B + s] = asg_pad[..., c*16*F_B + s*16 + w]."""
    A_pad = asg_pad.shape[-1]
    F_Bl = A_pad // 128
    v = asg_pad.reshape(*asg_pad.shape[:-1], 8, F_Bl, 16)     # [.., c, s, w]
    return np.ascontiguousarray(np.swapaxes(v, -1, -2)).reshape(
        *asg_pad.shape[:-1], A_pad)


def pad_inputs(dt, anchors, assign):
    pad = A_PAD - A
    dtp = np.pad(dt, ((0, 0), (0, 0), (0, pad)), constant_values=0.5)
    ancp = np.concatenate(
        [anchors, np.tile(np.array([[0, 0, 1, 1]], np.float32), (pad, 1))], 0)
    asgp = np.pad(assign, ((0, 0), (0, pad)), constant_values=-1)
    return (dtp.astype(np.float32, copy=False),
            ancp.astype(np.float32, copy=False),
            asgp.astype(np.int32, copy=False))


def host_combine(parts, ncol):
    parts = parts.reshape(-1, ncol).astype(np.float64)
    cls = (parts[:, COL_CLS0] + parts[:, COL_CLS0 + 1]
           + parts[:, COL_PC0] + parts[:, COL_PC0 + 1]
           + parts[:, COL_OBJ0] + parts[:, COL_OBJ0 + 1])
    box = (parts[:, COL_RELUP:COL_RELUP + 4].sum(1)
           + parts[:, COL_RELUN:COL_RELUN + 4].sum(1)
           + (0.5 / BETA) * parts[:, COL_MSQ:COL_MSQ + 4].sum(1))
    npos = np.maximum(parts[:, COL_NPOS0] + parts[:, COL_NPOS0 + 1], 1.0)
    return np.float32(np.sum((cls + box) / npos) / B)


_prog_cache = {}


def make_in_maps(inputs):
    dt, gt, anchors, assign = (inputs["dt"], inputs["gt"], inputs["anchors"],
                               inputs["assign"])
    dtp, ancp, asgp = pad_inputs(np.asarray(dt), np.asarray(anchors),
                                 np.asarray(assign))
    asg_il = interleave_assign(asgp)
    gtf = np.asarray(gt).astype(np.float32, copy=False)
    ones_dn, ones_up, iota_cls, ones_col = make_consts()
    in_maps = []
    for c in range(NCORES):
        sl = slice(c * BPC, (c + 1) * BPC)
        in_maps.append({
            "dtc": np.ascontiguousarray(dtp[sl]),
            "gtc": np.ascontiguousarray(gtf[sl]),
            "anc": ancp,
            "asg": np.ascontiguousarray(asgp[sl]),
            "asg_il": np.ascontiguousarray(asg_il[sl]),
            "ones_dn": ones_dn, "ones_up": ones_up,
            "iota_cls": iota_cls, "ones_col": ones_col,
        })
    return in_maps


def kernel(dt, gt, anchors, assign):
    from concourse.bass_utils import run_bass_kernel_spmd

    cfg = Cfg(A_PAD)
    if "nc" not in _prog_cache:
        _prog_cache["nc"] = build_program(cfg)
    nc = _prog_cache["nc"]

    in_maps = make_in_maps({"dt": dt, "gt": gt, "anchors": anchors,
                            "assign": assign})
    results = run_bass_kernel_spmd(nc, in_maps, core_ids=list(range(NCORES))).results
    parts = np.stack([results[c]["out"] for c in range(NCORES)])
    return host_combine(parts, cfg.NCOL)



# revision 7
# speedup vs baseline: 2.3413x; 2.3413x over previous
"""Trainium2 Bass kernel for nn_Defaultloss_49873160241482 (focal-BCE + smooth-L1 detection loss).

Self-contained: kernel(**inputs) takes full unsharded inputs, shards the batch
dim across 8 NeuronCores (2 batches/core), runs the Bass program via
run_bass_kernel_spmd, and combines per-core partial sums on the host.

v2 design (all heavy math in bf16, rel tolerance is 2e-2):
  - dt is cast to bf16 on the host; anchors are sent transposed [4, A] so
    every DMA line is contiguous. A = 120000 = 120x1000 box tiles and
    126x500 cls tiles exactly (no padding).
  - gt-row gather runs on gpsimd ap_gather with anchor PAIRING: one int16
    index g0*64+g1 per anchor pair, bf16 pair tables (d=2 -> one 32-bit
    word per index), so the Q7 inner loop runs half as many indices.
  - cls: per tile [126,500] (6 k-groups x 21 channels), phase 1 computes
    prod = 0.75 p^2 ln(1-p) (ACT Square/Ln -> DVE mult) and reduces over
    channels with a bf16 matmul against a -1 pack matrix (sliding window
    keeps per-anchor granularity in psum rows). Phase 2 broadcasts the
    per-anchor poisoned class id over the 21 channel partitions with a
    stride-0 DMA, selects p at the class channel, and reduces with a
    second bf16 matmul. p stays resident in SBUF between phases so dt is
    read exactly once.
  - masks/npos and the smooth-L1 box path run in box layout [120,1000].
  Device returns per-batch partial sums; host divides by n_pos and B.
"""

import numpy as np
import ml_dtypes

import concourse.bass as bass
import concourse.bacc as bacc
import concourse.mybir as mybir
import concourse.tile as tile

F32 = mybir.dt.float32
BF16 = mybir.dt.bfloat16
I16 = mybir.dt.int16
AF = mybir.ActivationFunctionType
OP = mybir.AluOpType

B, A, C, G = 16, 120000, 20, 64
ALPHA, GAMMA, BETA = 0.25, 2.0, 1.0 / 9.0
NCORES = 8
BPC = B // NCORES  # batches per core

NCH = 21          # 1 obj + 20 class channels (dt rows 4..24)
P_B, F_B = 120, 1000   # box layout
F_C = 500              # cls tile free size
T_C = 40               # cls tiles per batch (A/6/F_C)
G_T = 20               # tiles per psum accumulation group
NG = 2                 # groups
NPAIR = A // 2 // 8    # 7500 anchor pairs per Q7 core
NIDX = 7504            # padded to a multiple of 16
IDXC = NIDX // 16      # idx columns per partition

NCOL = 24
COL_CLS0 = 0      # + group g: sum L0 * (assign>=0)
COL_NPOS0 = 2
COL_PC0 = 4       # + g: sum Delta(pc)*(assign>=1) (missing 0.75 factor)
COL_OBJ0 = 6      # + g: same for obj channel
COL_RELUP = 8     # + box channel j
COL_RELUN = 12    # + j
COL_MSQ = 16      # + j

SQRT_075 = float(np.sqrt(0.75))


def _register_const_aps(nc, values):
    for value in values:
        t = nc.alloc_sbuf_tensor(f"const-f32-{value}", [128, 1], F32)
        nc.gpsimd.memset(t.ap(), value)
        nc.const_aps.aps[(F32, value)] = t.ap()
    nc.all_engine_barrier()


def build_program():
    nc = bacc.Bacc("TRN2", target_bir_lowering=False, debug=False)
    _register_const_aps(nc, [-BETA, 1.0])

    dtc = nc.dram_tensor("dtc", [BPC, 5 + C, A], BF16, kind="ExternalInput")
    anc = nc.dram_tensor("anc", [4, A], F32, kind="ExternalInput")
    asgbx = nc.dram_tensor("asgbx", [BPC, A], BF16, kind="ExternalInput")
    asgpk = nc.dram_tensor("asgpk", [BPC, NG, P_B, F_C], BF16, kind="ExternalInput")
    idxD = nc.dram_tensor("idxD", [BPC, 128, IDXC], I16, kind="ExternalInput")
    tblD = nc.dram_tensor("tblD", [BPC, 5, 8, 8192], BF16, kind="ExternalInput")
    kdnD = nc.dram_tensor("kdnD", [126, 366], BF16, kind="ExternalInput")
    iotaD = nc.dram_tensor("iotaD", [126, 1], F32, kind="ExternalInput")
    onesD = nc.dram_tensor("onesD", [P_B, 1], F32, kind="ExternalInput")
    out = nc.dram_tensor("out", [BPC, NCOL], F32, kind="ExternalOutput")

    with tile.TileContext(nc) as tc:
        with (
            tc.tile_pool(name="const", bufs=1) as cpool,
            tc.tile_pool(name="anch", bufs=1) as apool,
            tc.tile_pool(name="gat", bufs=1) as gpool,
            tc.tile_pool(name="gout", bufs=1) as gopool,
            tc.tile_pool(name="box", bufs=2) as bpool,
            tc.tile_pool(name="cls", bufs=2) as clspool,
            tc.tile_pool(name="pres", bufs=1) as prespool,
            tc.tile_pool(name="pk", bufs=1) as pkpool,
            tc.tile_pool(name="small", bufs=1) as spool,
            tc.tile_pool(name="ps_s0", bufs=2, space="PSUM") as ps0pool,
            tc.tile_pool(name="ps_pc", bufs=2, space="PSUM") as pspcool,
            tc.tile_pool(name="ps_fin", bufs=2, space="PSUM") as psfpool,
        ):
            ks = {}
            kdn = cpool.tile([126, 366], BF16, tag="kdn")
            nc.sync.dma_start(out=kdn[:], in_=kdnD[:, :])
            iota = cpool.tile([126, 1], F32, tag="iota")
            nc.sync.dma_start(out=iota[:], in_=iotaD[:, :])
            kones = cpool.tile([P_B, 1], F32, tag="kones")
            nc.sync.dma_start(out=kones[:], in_=onesD[:, :])
            ks.update(kdn=kdn, iota=iota, kones=kones)

            # ---- anchor precompute (shared by both batches), box layout ----
            an = {}
            for nm in ("iaw", "iah", "axw", "ayh", "lnaw", "lnah"):
                an[nm] = apool.tile([P_B, F_B], BF16, tag=nm, name=nm)

            for lo, hi, ia, lna, acw in (
                (0, 2, "iaw", "lnaw", "axw"), (1, 3, "iah", "lnah", "ayh")):
                sA = apool.tile([P_B, F_B], F32, tag="sA", name=f"sA_{lo}")
                sB = apool.tile([P_B, F_B], F32, tag="sB", name=f"sB_{lo}")
                nc.sync.dma_start(out=sA[:], in_=anc[lo].rearrange("(p f) -> p f", p=P_B))
                nc.sync.dma_start(out=sB[:], in_=anc[hi].rearrange("(p f) -> p f", p=P_B))
                nc.vector.tensor_tensor(sB[:], sB[:], sA[:], OP.subtract)   # aw f32
                rec = apool.tile([P_B, F_B], F32, tag="rec", name=f"rec_{lo}")
                nc.vector.reciprocal_approx_fast(rec[:], sB[:])             # 1/aw
                nc.vector.tensor_copy(an[ia][:], rec[:])                    # bf16
                nc.scalar.activation(an[lna][:], sB[:], AF.Ln)              # ln aw -> bf16
                # a_cx = x1 + 0.5*aw ; axw = a_cx / aw
                nc.vector.scalar_tensor_tensor(sA[:], sB[:], 0.5, sA[:],
                                               OP.mult, OP.add)             # a_c f32
                nc.vector.tensor_tensor(sA[:], sA[:], rec[:], OP.mult)      # a_c/aw f32
                nc.vector.tensor_copy(an[acw][:], sA[:])                    # bf16

            for b in range(BPC):
                build_batch(nc, tc, b, dtc=dtc, asgbx=asgbx, asgpk=asgpk,
                            idxD=idxD, tblD=tblD, out=out, ks=ks, an=an,
                            gpool=gpool, gopool=gopool, bpool=bpool,
                            clspool=clspool, prespool=prespool, pkpool=pkpool,
                            spool=spool, ps0pool=ps0pool, pspcool=pspcool,
                            psfpool=psfpool)

    nc.compile()
    return nc


def build_batch(nc, tc, b, *, dtc, asgbx, asgpk, idxD, tblD, out, ks, an,
                gpool, gopool, bpool, clspool, prespool, pkpool, spool,
                ps0pool, pspcool, psfpool):
    kdn, iota, kones = ks["kdn"], ks["iota"], ks["kones"]

    strip = spool.tile([P_B, NCOL], F32, tag="strip")
    nc.vector.memset(strip[:], 0.0)

    # ---------- gather inputs ----------
    idx_t = gpool.tile([128, IDXC], I16, tag="idx")
    nc.sync.dma_start(out=idx_t[:], in_=idxD[b, :, :])
    tblT = gpool.tile([128, 8192], BF16, tag="tbl")
    for r in range(5):
        nc.sync.dma_start(out=tblT[r::16, :], in_=tblD[b, r])

    # ---------- assign masks / npos ----------
    asgb = bpool.tile([P_B, F_B], BF16, tag="asgb")
    nc.sync.dma_start(out=asgb[:], in_=asgbx[b, :].rearrange("(p f) -> p f", p=P_B))
    m1 = bpool.tile([P_B, F_B], BF16, tag="m1")
    nc.vector.tensor_scalar(m1[:], asgb[:], 1.0, None, OP.is_ge, OP.add,
                            accum_out=strip[:, COL_NPOS0:COL_NPOS0 + 1])

    # ---------- ap_gather: pairs of anchors, bf16 pair tables ----------
    gout = gopool.tile([128, 2 * NIDX], BF16, tag="gout", name=f"gout_{b}")
    nc.gpsimd.ap_gather(
        out_ap=gout[:].rearrange("p (n d) -> p n d", d=2),
        in_ap=tblT[:].rearrange("p (n d) -> p n d", d=2),
        idxs_ap=idx_t[:],
        channels=128, num_elems=4096, d=2, num_idxs=NIDX,
    )
    fld = {}
    for r, nm in enumerate(("Gx", "Gy", "LNw", "LNh", "CLS")):
        fld[nm] = bpool.tile([P_B, F_B], BF16, tag=f"fld_{nm}", name=f"fld_{nm}_{b}")
        for c in range(8):
            nc.sync.dma_start(
                out=fld[nm][15 * c:15 * (c + 1), :],
                in_=gout[16 * c + r:16 * c + r + 1, :2 * NPAIR].rearrange(
                    "p (q f) -> p q f", q=15),
            )

    # ---------- cls phase 1: prod = 0.75 p^2 ln(1-p), matmul channel-sum ----
    dview = dtc[b, 4:, :].rearrange("c (k t f) -> t k c f", k=6, f=F_C)
    ps_S0 = [ps0pool.tile([126, F_C], F32, tag="ps_S0", name=f"ps_S0_{g}_{b}")
             for g in range(NG)]
    pres = [prespool.tile([126, 10 * F_C], BF16, tag=f"pres{s}", name=f"pres{s}")
            for s in range(4)]
    for t in range(T_C):
        g, tl = t // G_T, t % G_T
        psl = pres[t // 10][:, (t % 10) * F_C:(t % 10 + 1) * F_C]
        nc.sync.dma_start(out=psl, in_=dview[t])
        sq = clspool.tile([126, F_C], BF16, tag="sq")
        nc.scalar.activation(sq[:], psl, AF.Square, scale=SQRT_075)
        ln1m = clspool.tile([126, F_C], BF16, tag="ln1m")
        nc.scalar.activation(ln1m[:], psl, AF.Ln, bias=1.0, scale=-1.0)
        prod = clspool.tile([126, F_C], BF16, tag="prod")
        nc.vector.tensor_tensor(prod[:], sq[:], ln1m[:], OP.mult)
        nc.tensor.matmul(ps_S0[g][:, :], lhsT=kdn[:, 240 - 6 * tl:366 - 6 * tl],
                         rhs=prod[:], start=(tl == 0), stop=(tl == G_T - 1))

    # packed S0 reduction per group (mask assign>=0)
    asg_pk = [pkpool.tile([P_B, F_C], BF16, tag="asg_pk", name=f"asg_pk_{g}_{b}")
              for g in range(NG)]
    for g in range(NG):
        nc.sync.dma_start(out=asg_pk[g][:], in_=asgpk[b, g])
        jk = pkpool.tile([P_B, F_C], BF16, tag="jk")
        nc.vector.scalar_tensor_tensor(
            jk[:], asg_pk[g][:], 0.0, ps_S0[g][:P_B, :], OP.is_ge, OP.mult,
            accum_out=strip[:, COL_CLS0 + g:COL_CLS0 + g + 1])

    # ---------- poisoned class id (box layout) ----------
    clsP = bpool.tile([P_B, F_B], BF16, tag="clsP", name=f"clsP_{b}")
    nc.vector.tensor_tensor(clsP[:], fld["CLS"][:], m1[:], OP.mult)
    nc.vector.tensor_scalar(clsP[:], clsP[:], 7.0, None, OP.subtract)

    # ---------- box path ----------
    for j, (Gc, ia, acw) in enumerate(
        [(fld["Gx"], an["iaw"], an["axw"]), (fld["Gy"], an["iah"], an["ayh"])]
    ):
        t1 = bpool.tile([P_B, F_B], BF16, tag="bx_t1")
        nc.vector.tensor_tensor(t1[:], Gc[:], ia[:], OP.mult)
        dl = bpool.tile([P_B, F_B], BF16, tag="bx_dl")
        nc.sync.dma_start(out=dl[:], in_=dtc[b, j, :].rearrange("(p f) -> p f", p=P_B))
        cxy = bpool.tile([P_B, F_B], BF16, tag="bx_c")
        nc.vector.tensor_tensor(cxy[:], acw[:], dl[:], OP.add)
        dm = bpool.tile([P_B, F_B], BF16, tag="bx_d", name=f"bx_d{j}_{b}")
        nc.vector.tensor_tensor(dm[:], t1[:], cxy[:], OP.subtract)
        box_sl1(nc, bpool, dm, m1, strip, j)
    for j, (Lc, lna) in enumerate([(fld["LNw"], an["lnaw"]), (fld["LNh"], an["lnah"])]):
        dl = bpool.tile([P_B, F_B], BF16, tag="bx_dl")
        nc.sync.dma_start(out=dl[:],
                          in_=dtc[b, 2 + j, :].rearrange("(p f) -> p f", p=P_B))
        cwh = bpool.tile([P_B, F_B], BF16, tag="bx_c")
        nc.vector.tensor_tensor(cwh[:], lna[:], dl[:], OP.add)
        dm = bpool.tile([P_B, F_B], BF16, tag="bx_d", name=f"bx_d{2+j}_{b}")
        nc.vector.tensor_tensor(dm[:], Lc[:], cwh[:], OP.subtract)
        box_sl1(nc, bpool, dm, m1, strip, 2 + j)

    # ---------- cls phase 2: class-channel select + matmul ----------
    ps_pc = [pspcool.tile([126, F_C], F32, tag="ps_pc", name=f"ps_pc_{g}_{b}")
             for g in range(NG)]
    for t in range(T_C):
        g, tl = t // G_T, t % G_T
        psl = pres[t // 10][:, (t % 10) * F_C:(t % 10 + 1) * F_C]
        rep = clspool.tile([126, F_C], BF16, tag="rep")
        src = clsP[(t // 2)::20, (t % 2) * F_C:(t % 2 + 1) * F_C]
        nc.sync.dma_start(out=rep[:], in_=src.unsqueeze(1).to_broadcast((6, NCH, F_C)))
        e = clspool.tile([126, F_C], BF16, tag="e")
        nc.vector.tensor_scalar(e[:], rep[:], iota[:, 0:1], 0.0,
                                OP.subtract, OP.is_equal)
        z = clspool.tile([126, F_C], BF16, tag="z")
        nc.vector.tensor_tensor(z[:], e[:], psl, OP.mult)
        nc.tensor.matmul(ps_pc[g][:, :], lhsT=kdn[:, 240 - 6 * tl:366 - 6 * tl],
                         rhs=z[:], start=(tl == 0), stop=(tl == G_T - 1))

    # packed pc/obj delta reductions
    d4view = dtc[b, 4, :].rearrange("(k t f) -> t k f", k=6, f=F_C)
    for g in range(NG):
        fill = pkpool.tile([P_B, F_C], BF16, tag="fill")
        nc.vector.tensor_scalar(fill[:], asg_pk[g][:], 1.0, 0.5, OP.is_lt, OP.mult)
        pc = pkpool.tile([P_B, F_C], BF16, tag="pc")
        nc.vector.tensor_tensor(pc[:], fill[:], ps_pc[g][:P_B, :], OP.subtract)
        delta_masked_sum(nc, pkpool, pc, asg_pk[g],
                         strip[:, COL_PC0 + g:COL_PC0 + g + 1])
        p0 = pkpool.tile([P_B, F_C], BF16, tag="p0")
        nc.sync.dma_start(out=p0[:], in_=d4view[g * G_T:(g + 1) * G_T])
        delta_masked_sum(nc, pkpool, p0, asg_pk[g],
                         strip[:, COL_OBJ0 + g:COL_OBJ0 + g + 1])

    # ---------- finalize ----------
    ps_fin = psfpool.tile([1, NCOL], F32, tag="ps_fin")
    nc.tensor.matmul(ps_fin[:], lhsT=kones[:], rhs=strip[:], start=True, stop=True)
    fin = spool.tile([1, NCOL], F32, tag="fin")
    nc.vector.tensor_copy(fin[:], ps_fin[:])
    nc.sync.dma_start(out=out[b, :].unsqueeze(0), in_=fin[:])


def box_sl1(nc, pool, d, m1, strip, j):
    dm = d  # poison in place: d * (assign>=1)
    nc.vector.tensor_tensor(dm[:], d[:], m1[:], OP.mult)
    junk = pool.tile([P_B, F_B], BF16, tag="junk")
    nc.scalar.activation(junk[:], dm[:], AF.Relu, bias=-BETA, scale=1.0,
                         accum_out=strip[:, COL_RELUP + j:COL_RELUP + j + 1])
    junk2 = pool.tile([P_B, F_B], BF16, tag="junk")
    nc.scalar.activation(junk2[:], dm[:], AF.Relu, bias=-BETA, scale=-1.0,
                         accum_out=strip[:, COL_RELUN + j:COL_RELUN + j + 1])
    sqd = pool.tile([P_B, F_B], BF16, tag="sqd")
    nc.scalar.activation(sqd[:], dm[:], AF.Square)
    junk3 = pool.tile([P_B, F_B], BF16, tag="junk")
    nc.vector.tensor_scalar(junk3[:], sqd[:], BETA * BETA, 0.0, OP.min, OP.add,
                            accum_out=strip[:, COL_MSQ + j:COL_MSQ + j + 1])


def delta_masked_sum(nc, pool, pv, asg_pk, acc_col):
    """acc_col = sum_free (asg>=1) * [Delta(pv)/0.75];
    Delta(p) = -0.25*(1-p)^2*ln(p) + 0.75*p^2*ln(1-p); host multiplies 0.75."""
    lnp = pool.tile([P_B, F_C], BF16, tag="dc_lnp")
    nc.scalar.activation(lnp[:], pv[:], AF.Ln)
    ln1mp = pool.tile([P_B, F_C], BF16, tag="dc_ln1mp")
    nc.scalar.activation(ln1mp[:], pv[:], AF.Ln, bias=1.0, scale=-1.0)
    sqp = pool.tile([P_B, F_C], BF16, tag="dc_sqp")
    nc.scalar.activation(sqp[:], pv[:], AF.Square)
    sq1mp = pool.tile([P_B, F_C], BF16, tag="dc_sq1mp")
    nc.scalar.activation(sq1mp[:], pv[:], AF.Square, bias=1.0, scale=-1.0)
    t1 = pool.tile([P_B, F_C], BF16, tag="dc_t1")
    nc.vector.tensor_tensor(t1[:], sq1mp[:], lnp[:], OP.mult)     # (1-p)^2 ln p
    t2 = pool.tile([P_B, F_C], BF16, tag="dc_t2")
    nc.vector.tensor_tensor(t2[:], sqp[:], ln1mp[:], OP.mult)     # p^2 ln(1-p)
    dd = pool.tile([P_B, F_C], BF16, tag="dc_dd")
    nc.vector.scalar_tensor_tensor(dd[:], t1[:], -1.0 / 3.0, t2[:], OP.mult, OP.add)
    jk = pool.tile([P_B, F_C], BF16, tag="dc_jk")
    nc.vector.scalar_tensor_tensor(jk[:], asg_pk[:], 1.0, dd[:], OP.is_ge, OP.mult,
                                   accum_out=acc_col)


# ======================= host-side prep =======================

def make_consts():
    # sliding-window pack matrix (entries -1): cls tile tl uses
    # kdn[:, 240-6*tl : 366-6*tl], which has -1 at [p, 6*tl + p//21].
    kdn = np.zeros((126, 366), ml_dtypes.bfloat16)
    for p in range(126):
        kdn[p, 240 + p // NCH] = -1.0
    iota = (np.arange(126, dtype=np.float32) % NCH - 1.0).reshape(126, 1)
    ones_col = np.ones((P_B, 1), np.float32)
    return kdn, iota, ones_col


def make_pair_tables(gt):
    """[B, 5, 8, 8192] bf16: row r replicated 8x; entry (g0*64+g1) holds the
    (f[g0], f[g1]) bf16 pair for field r in [Gx, Gy, ln w, ln h, cls+6]."""
    gt = np.asarray(gt, np.float32)  # [B, G, 5]
    gx = gt[:, :, 0] + 0.5 * gt[:, :, 2]
    gy = gt[:, :, 1] + 0.5 * gt[:, :, 3]
    lnw = np.log(gt[:, :, 2])
    lnh = np.log(gt[:, :, 3])
    cl = gt[:, :, 4] + 6.0  # cls - 1 + 7
    f = np.stack([gx, gy, lnw, lnh, cl], axis=1)  # [B, 5, G]
    g0 = np.arange(4096) // 64
    g1 = np.arange(4096) % 64
    pair = np.stack([f[:, :, g0], f[:, :, g1]], axis=-1)  # [B, 5, 4096, 2]
    pair = pair.reshape(B, 5, 1, 8192).astype(ml_dtypes.bfloat16)
    return np.ascontiguousarray(np.broadcast_to(pair, (B, 5, 8, 8192)))


def make_pair_idx(assign):
    """[B, 128, IDXC] int16: pair index g0*64+g1 per anchor pair, wrapped
    16-partition round-robin per Q7 core (core c owns anchors [15000c,...))."""
    gidx = np.clip(np.asarray(assign, np.int64) - 1, 0, G - 1)
    pair = (gidx[:, 0::2] * 64 + gidx[:, 1::2]).astype(np.int16)  # [B, A/2]
    pair = pair.reshape(B, 8, NPAIR)
    pad = np.zeros((B, 8, NIDX - NPAIR), np.int16)
    pair = np.concatenate([pair, pad], axis=2)          # [B, 8, NIDX]
    pair = pair.reshape(B, 8, IDXC, 16)
    pair = np.swapaxes(pair, 2, 3)                      # [B, 8, 16, IDXC]
    return np.ascontiguousarray(pair.reshape(B, 128, IDXC))


def make_asg_packed(assign):
    """[B, NG, 120, 500] bf16 in the cls packed layout (row 6*tl+k)."""
    a = np.asarray(assign, np.float32).reshape(B, 6, NG, G_T, F_C)
    a = np.transpose(a, (0, 2, 3, 1, 4))  # [B, NG, tl, k, f]
    return np.ascontiguousarray(a.reshape(B, NG, P_B, F_C).astype(ml_dtypes.bfloat16))


def host_combine(parts):
    parts = parts.reshape(-1, NCOL).astype(np.float64)
    cls = (parts[:, COL_CLS0] + parts[:, COL_CLS0 + 1]
           + 0.75 * (parts[:, COL_PC0] + parts[:, COL_PC0 + 1]
                     + parts[:, COL_OBJ0] + parts[:, COL_OBJ0 + 1]))
    box = (parts[:, COL_RELUP:COL_RELUP + 4].sum(1)
           + parts[:, COL_RELUN:COL_RELUN + 4].sum(1)
           + (0.5 / BETA) * parts[:, COL_MSQ:COL_MSQ + 4].sum(1))
    npos = np.maximum(parts[:, COL_NPOS0] + parts[:, COL_NPOS0 + 1], 1.0)
    return np.float32(np.sum((cls + box) / npos) / B)


def make_in_maps(inputs):
    dt = np.asarray(inputs["dt"], np.float32)
    gt = np.asarray(inputs["gt"], np.float32)
    anchors = np.asarray(inputs["anchors"], np.float32)
    assign = np.asarray(inputs["assign"])
    dtb = dt.astype(ml_dtypes.bfloat16)
    ancT = np.ascontiguousarray(anchors.T)  # [4, A]
    asgb = assign.astype(ml_dtypes.bfloat16)
    asgpk = make_asg_packed(assign)
    idx = make_pair_idx(assign)
    tbl = make_pair_tables(gt)
    kdn, iota, ones_col = make_consts()
    in_maps = []
    for c in range(NCORES):
        sl = slice(c * BPC, (c + 1) * BPC)
        in_maps.append({
            "dtc": np.ascontiguousarray(dtb[sl]),
            "anc": ancT,
            "asgbx": np.ascontiguousarray(asgb[sl]),
            "asgpk": np.ascontiguousarray(asgpk[sl]),
            "idxD": np.ascontiguousarray(idx[sl]),
            "tblD": np.ascontiguousarray(tbl[sl]),
            "kdnD": kdn, "iotaD": iota, "onesD": ones_col,
        })
    return in_maps


_prog_cache = {}


def kernel(dt, gt, anchors, assign):
    from concourse.bass_utils import run_bass_kernel_spmd

    if "nc" not in _prog_cache:
        _prog_cache["nc"] = build_program()
    nc = _prog_cache["nc"]

    in_maps = make_in_maps({"dt": dt, "gt": gt, "anchors": anchors,
                            "assign": assign})
    results = run_bass_kernel_spmd(nc, in_maps, core_ids=list(range(NCORES))).results
    parts = np.stack([results[c]["out"] for c in range(NCORES)])
    return host_combine(parts)


# revision 9
# speedup vs baseline: 2.4458x; 1.0447x over previous
"""Trainium2 Bass kernel for nn_Defaultloss_49873160241482 (focal-BCE + smooth-L1 detection loss).

Self-contained: kernel(**inputs) takes full unsharded inputs, shards the batch
dim across 8 NeuronCores (2 batches/core), runs the Bass program via
run_bass_kernel_spmd, and combines per-core partial sums on the host.

v3 design (all heavy math in bf16, rel tolerance is 2e-2):
  - dt is cast to bf16 on the host and relaid out so every DMA is a handful
    of big regular descriptors: box rows [4, A], cls rows as four
    [126, 5000] tile-major slabs per batch, obj row also packed [NG,120,500].
  - gt-row gather uses anchor PAIRING (index g0*64+g1, bf16 pair tables):
    either gpsimd ap_gather (one call/batch) or chunked native IndirectCopy.
  - cls phase 1 (gather-independent): prod = 0.75 p^2 ln(1-p) via two
    slab-wide ACTs + one DVE mult, channel-reduced by bf16 matmuls against
    a -1 pack matrix (sliding window keeps per-anchor sums in psum rows).
    p stays resident in SBUF. Phase 2 (after gather): poisoned class id is
    broadcast over the 21 channel partitions via a stride-0 DRAM re-read,
    one-hot select, second bf16 matmul chain.
  - masks/npos and the smooth-L1 box path run in box layout [120,1000].
  Device returns per-batch partial sums; host divides by n_pos and B.
"""

import numpy as np
import ml_dtypes

import concourse.bass as bass
import concourse.bacc as bacc
import concourse.mybir as mybir
import concourse.tile as tile

F32 = mybir.dt.float32
BF16 = mybir.dt.bfloat16
I16 = mybir.dt.int16
U16 = mybir.dt.uint16
AF = mybir.ActivationFunctionType
OP = mybir.AluOpType

B, A, C, G = 16, 120000, 20, 64
ALPHA, GAMMA, BETA = 0.25, 2.0, 1.0 / 9.0
NCORES = 8
BPC = B // NCORES  # batches per core

GATHER_KIND = "ap"  # "ap" (gpsimd ap_gather) or "ic" (native IndirectCopy)

NCH = 21          # 1 obj + 20 class channels (dt rows 4..24)
P_B, F_B = 120, 1000   # box layout
F_C = 500              # cls tile free size
T_C = 40               # cls tiles per batch (A/6/F_C)
G_T = 20               # tiles per psum accumulation group
NG = 2                 # groups
NSLAB = 4              # phase slabs of 10 tiles
NPAIR = A // 2 // 8    # 7500 anchor pairs per Q7 core
NIDX = 7680            # padded to 15*512 for IC chunking (and mult of 16)
IDXC = NIDX // 16      # idx columns per partition
IC_CHUNK = 512         # pairs per IndirectCopy (dst elem count <= 1024)

NCOL = 24
COL_CLS0 = 0      # + group g: sum L0 * (assign>=0)
COL_NPOS0 = 2
COL_PC0 = 4       # + g: sum Delta(pc)*(assign>=1) (missing 0.75 factor)
COL_OBJ0 = 6      # + g: same for obj channel
COL_RELUP = 8     # + box channel j
COL_RELUN = 12    # + j
COL_MSQ = 16      # + j

SQRT_075 = float(np.sqrt(0.75))


def _register_const_aps(nc, values):
    for value in values:
        t = nc.alloc_sbuf_tensor(f"const-f32-{value}", [128, 1], F32)
        nc.gpsimd.memset(t.ap(), value)
        nc.const_aps.aps[(F32, value)] = t.ap()
    nc.all_engine_barrier()


def build_program():
    nc = bacc.Bacc("TRN2", target_bir_lowering=False, debug=False)
    _register_const_aps(nc, [-BETA, 1.0])

    dtbox = nc.dram_tensor("dtbox", [BPC, 4, A], BF16, kind="ExternalInput")
    dtcls = nc.dram_tensor("dtcls", [BPC, NSLAB, 126, 10 * F_C], BF16,
                           kind="ExternalInput")
    d4pk = nc.dram_tensor("d4pk", [BPC, NG, P_B, F_C], BF16, kind="ExternalInput")
    anc = nc.dram_tensor("anc", [4, A], F32, kind="ExternalInput")
    asgbx = nc.dram_tensor("asgbx", [BPC, A], BF16, kind="ExternalInput")
    asgpk = nc.dram_tensor("asgpk", [BPC, NG, P_B, F_C], BF16, kind="ExternalInput")
    idx_dt = I16 if GATHER_KIND == "ap" else U16
    idxD = nc.dram_tensor("idxD", [BPC, 128, IDXC], idx_dt, kind="ExternalInput")
    tblD = nc.dram_tensor("tblD", [BPC, 5, 8, 8192], BF16, kind="ExternalInput")
    kdnD = nc.dram_tensor("kdnD", [126, 366], BF16, kind="ExternalInput")
    iotaD = nc.dram_tensor("iotaD", [126, 1], F32, kind="ExternalInput")
    onesD = nc.dram_tensor("onesD", [P_B, 1], F32, kind="ExternalInput")
    clsPD = nc.dram_tensor("clsPD", [BPC, A], BF16)
    out = nc.dram_tensor("out", [BPC, NCOL], F32, kind="ExternalOutput")

    with tile.TileContext(nc) as tc:
        with (
            tc.tile_pool(name="const", bufs=1) as cpool,
            tc.tile_pool(name="anch", bufs=1) as apool,
            tc.tile_pool(name="gat", bufs=1) as gpool,
            tc.tile_pool(name="gout", bufs=1) as gopool,
            tc.tile_pool(name="box", bufs=2) as bpool,
            tc.tile_pool(name="slab", bufs=1) as slpool,
            tc.tile_pool(name="repz", bufs=2) as rppool,
            tc.tile_pool(name="pres", bufs=1) as prespool,
            tc.tile_pool(name="pk", bufs=1) as pkpool,
            tc.tile_pool(name="small", bufs=1) as spool,
            tc.tile_pool(name="ps_s0", bufs=2, space="PSUM") as ps0pool,
            tc.tile_pool(name="ps_pc", bufs=2, space="PSUM") as pspcool,
            tc.tile_pool(name="ps_fin", bufs=2, space="PSUM") as psfpool,
        ):
            ks = {}
            kdn = cpool.tile([126, 366], BF16, tag="kdn")
            nc.sync.dma_start(out=kdn[:], in_=kdnD[:, :])
            iota = cpool.tile([126, 1], F32, tag="iota")
            nc.sync.dma_start(out=iota[:], in_=iotaD[:, :])
            kones = cpool.tile([P_B, 1], F32, tag="kones")
            nc.sync.dma_start(out=kones[:], in_=onesD[:, :])
            ks.update(kdn=kdn, iota=iota, kones=kones)

            # ---- anchor precompute (shared by both batches), box layout ----
            an = {}
            for nm in ("iaw", "iah", "axw", "ayh", "lnaw", "lnah"):
                an[nm] = apool.tile([P_B, F_B], BF16, tag=nm, name=nm)

            for lo, hi, ia, lna, acw in (
                (0, 2, "iaw", "lnaw", "axw"), (1, 3, "iah", "lnah", "ayh")):
                sA = apool.tile([P_B, F_B], F32, tag="sA", name=f"sA_{lo}")
                sB = apool.tile([P_B, F_B], F32, tag="sB", name=f"sB_{lo}")
                nc.sync.dma_start(out=sA[:], in_=anc[lo].rearrange("(p f) -> p f", p=P_B))
                nc.sync.dma_start(out=sB[:], in_=anc[hi].rearrange("(p f) -> p f", p=P_B))
                nc.vector.tensor_tensor(sB[:], sB[:], sA[:], OP.subtract)   # aw f32
                rec = apool.tile([P_B, F_B], F32, tag="rec", name=f"rec_{lo}")
                nc.vector.reciprocal_approx_fast(rec[:], sB[:])             # 1/aw
                nc.vector.tensor_copy(an[ia][:], rec[:])                    # bf16
                nc.scalar.activation(an[lna][:], sB[:], AF.Ln)              # ln aw -> bf16
                # a_cx = x1 + 0.5*aw ; axw = a_cx / aw
                nc.vector.scalar_tensor_tensor(sA[:], sB[:], 0.5, sA[:],
                                               OP.mult, OP.add)             # a_c f32
                nc.vector.tensor_tensor(sA[:], sA[:], rec[:], OP.mult)      # a_c/aw f32
                nc.vector.tensor_copy(an[acw][:], sA[:])                    # bf16

            for b in range(BPC):
                build_batch(nc, tc, b, dtbox=dtbox, dtcls=dtcls, d4pk=d4pk,
                            asgbx=asgbx, asgpk=asgpk, idxD=idxD, tblD=tblD,
                            clsPD=clsPD, out=out, ks=ks, an=an,
                            gpool=gpool, gopool=gopool, bpool=bpool,
                            slpool=slpool, rppool=rppool, prespool=prespool,
                            pkpool=pkpool, spool=spool, ps0pool=ps0pool,
                            pspcool=pspcool, psfpool=psfpool)

    nc.compile()
    return nc


def build_batch(nc, tc, b, *, dtbox, dtcls, d4pk, asgbx, asgpk, idxD, tblD,
                clsPD, out, ks, an, gpool, gopool, bpool, slpool, rppool,
                prespool, pkpool, spool, ps0pool, pspcool, psfpool):
    kdn, iota, kones = ks["kdn"], ks["iota"], ks["kones"]

    strip = spool.tile([P_B, NCOL], F32, tag="strip")
    nc.vector.memset(strip[:], 0.0)

    # ---------- gather inputs ----------
    idx_t = gpool.tile([128, IDXC], I16 if GATHER_KIND == "ap" else U16, tag="idx")
    nc.sync.dma_start(out=idx_t[:], in_=idxD[b, :, :])
    tblT = gpool.tile([128, 8192], BF16, tag="tbl")
    for r in range(5):
        nc.sync.dma_start(out=tblT[r::16, :], in_=tblD[b, r])

    # ---------- assign masks / npos ----------
    asgb = bpool.tile([P_B, F_B], BF16, tag="asgb")
    nc.sync.dma_start(out=asgb[:], in_=asgbx[b, :].rearrange("(p f) -> p f", p=P_B))
    m1 = bpool.tile([P_B, F_B], BF16, tag="m1")
    nc.vector.tensor_scalar(m1[:], asgb[:], 1.0, None, OP.is_ge, OP.add,
                            accum_out=strip[:, COL_NPOS0:COL_NPOS0 + 1])

    # ---------- gather: pairs of anchors, bf16 pair tables ----------
    gout = gopool.tile([128, 2 * NIDX], BF16, tag="gout", name=f"gout_{b}")
    if GATHER_KIND == "ap":
        nc.gpsimd.ap_gather(
            out_ap=gout[:].rearrange("p (n d) -> p n d", d=2),
            in_ap=tblT[:].rearrange("p (n d) -> p n d", d=2),
            idxs_ap=idx_t[:],
            channels=128, num_elems=4096, d=2, num_idxs=NIDX,
        )
    else:
        gview = gout[:].rearrange("p (n d) -> p n d", d=2)
        for k in range(NIDX // IC_CHUNK):
            nc.gpsimd.indirect_copy(
                out=gview[:, k * IC_CHUNK:(k + 1) * IC_CHUNK],
                data=tblT[:].rearrange("p (n d) -> p n d", d=2),
                idxs=idx_t[:, k * (IC_CHUNK // 16):(k + 1) * (IC_CHUNK // 16)],
                i_know_ap_gather_is_preferred=True)
    fld = {}
    for r, nm in enumerate(("Gx", "Gy", "LNw", "LNh", "CLS")):
        fld[nm] = bpool.tile([P_B, F_B], BF16, tag=f"fld_{nm}", name=f"fld_{nm}_{b}")
        for c in range(8):
            nc.sync.dma_start(
                out=fld[nm][15 * c:15 * (c + 1), :],
                in_=gout[16 * c + r:16 * c + r + 1, :2 * NPAIR].rearrange(
                    "p (q f) -> p q f", q=15),
            )

    # ---------- cls phase 1: prod = 0.75 p^2 ln(1-p), matmul channel-sum ----
    ps_S0 = [ps0pool.tile([126, F_C], F32, tag="ps_S0", name=f"ps_S0_{g}_{b}")
             for g in range(NG)]
    pres = [prespool.tile([126, 10 * F_C], BF16, tag=f"pres{s}", name=f"pres{s}")
            for s in range(NSLAB)]
    for s in range(NSLAB):
        nc.sync.dma_start(out=pres[s][:], in_=dtcls[b, s])
        sq = slpool.tile([126, 10 * F_C], BF16, tag="sq", name=f"sq_{b}_{s}")
        nc.scalar.activation(sq[:], pres[s][:], AF.Square, scale=SQRT_075)
        ln1m = rppool.tile([126, 10 * F_C], BF16, tag="rep", name=f"ln1m_{b}_{s}")
        nc.scalar.activation(ln1m[:], pres[s][:], AF.Ln, bias=1.0, scale=-1.0)
        nc.vector.tensor_tensor(sq[:], sq[:], ln1m[:], OP.mult)  # prod in place
        for i in range(10):
            t = 10 * s + i
            g, tl = t // G_T, t % G_T
            nc.tensor.matmul(ps_S0[g][:, :], lhsT=kdn[:, 240 - 6 * tl:366 - 6 * tl],
                             rhs=sq[:, i * F_C:(i + 1) * F_C],
                             start=(tl == 0), stop=(tl == G_T - 1))

    # packed S0 reduction per group (mask assign>=0)
    asg_pk = [pkpool.tile([P_B, F_C], BF16, tag=f"asg_pk{g}", name=f"asg_pk_{g}_{b}")
              for g in range(NG)]
    for g in range(NG):
        nc.sync.dma_start(out=asg_pk[g][:], in_=asgpk[b, g])
        jk = pkpool.tile([P_B, F_C], BF16, tag="jk")
        nc.vector.scalar_tensor_tensor(
            jk[:], asg_pk[g][:], 0.0, ps_S0[g][:P_B, :], OP.is_ge, OP.mult,
            accum_out=strip[:, COL_CLS0 + g:COL_CLS0 + g + 1])

    # ---------- poisoned class id -> DRAM for partition-broadcast ----------
    clsP = bpool.tile([P_B, F_B], BF16, tag="clsP", name=f"clsP_{b}")
    nc.vector.tensor_tensor(clsP[:], fld["CLS"][:], m1[:], OP.mult)
    nc.vector.tensor_scalar(clsP[:], clsP[:], 7.0, None, OP.subtract)
    nc.sync.dma_start(out=clsPD[b, :].rearrange("(p f) -> p f", p=P_B), in_=clsP[:])

    # ---------- box path ----------
    for j, (Gc, ia, acw) in enumerate(
        [(fld["Gx"], an["iaw"], an["axw"]), (fld["Gy"], an["iah"], an["ayh"])]
    ):
        t1 = bpool.tile([P_B, F_B], BF16, tag="bx_t1")
        nc.vector.tensor_tensor(t1[:], Gc[:], ia[:], OP.mult)
        dl = bpool.tile([P_B, F_B], BF16, tag="bx_dl")
        nc.sync.dma_start(out=dl[:], in_=dtbox[b, j, :].rearrange("(p f) -> p f", p=P_B))
        nc.vector.tensor_tensor(dl[:], acw[:], dl[:], OP.add)
        nc.vector.tensor_tensor(t1[:], t1[:], dl[:], OP.subtract)
        box_sl1(nc, bpool, t1, m1, strip, j)
    for j, (Lc, lna) in enumerate([(fld["LNw"], an["lnaw"]), (fld["LNh"], an["lnah"])]):
        dl = bpool.tile([P_B, F_B], BF16, tag="bx_dl")
        nc.sync.dma_start(out=dl[:],
                          in_=dtbox[b, 2 + j, :].rearrange("(p f) -> p f", p=P_B))
        nc.vector.tensor_tensor(dl[:], lna[:], dl[:], OP.add)
        t1 = bpool.tile([P_B, F_B], BF16, tag="bx_t1")
        nc.vector.tensor_tensor(t1[:], Lc[:], dl[:], OP.subtract)
        box_sl1(nc, bpool, t1, m1, strip, 2 + j)

    # ---------- cls phase 2: class-channel select + matmul ----------
    ps_pc = [pspcool.tile([126, F_C], F32, tag="ps_pc", name=f"ps_pc_{g}_{b}")
             for g in range(NG)]
    cls_view = clsPD[b, :].rearrange("(k t f) -> k t f", k=6, f=F_C)
    for s in range(NSLAB):
        rep = rppool.tile([126, 10 * F_C], BF16, tag="rep", name=f"rep_{b}_{s}")
        src = cls_view[:, 10 * s:10 * (s + 1), :].unsqueeze(1)
        nc.sync.dma_start(out=rep[:], in_=src.to_broadcast((6, NCH, 10, F_C)))
        nc.vector.tensor_scalar(rep[:], rep[:], iota[:, 0:1], 0.0,
                                OP.subtract, OP.is_equal)
        nc.vector.tensor_tensor(rep[:], rep[:], pres[s][:], OP.mult)
        for i in range(10):
            t = 10 * s + i
            g, tl = t // G_T, t % G_T
            nc.tensor.matmul(ps_pc[g][:, :], lhsT=kdn[:, 240 - 6 * tl:366 - 6 * tl],
                             rhs=rep[:, i * F_C:(i + 1) * F_C],
                             start=(tl == 0), stop=(tl == G_T - 1))

    # packed pc/obj delta reductions
    for g in range(NG):
        fill = pkpool.tile([P_B, F_C], BF16, tag="fill")
        nc.vector.tensor_scalar(fill[:], asg_pk[g][:], 1.0, 0.5, OP.is_lt, OP.mult)
        pc = pkpool.tile([P_B, F_C], BF16, tag="pc")
        nc.vector.tensor_tensor(pc[:], fill[:], ps_pc[g][:P_B, :], OP.subtract)
        delta_masked_sum(nc, pkpool, pc, asg_pk[g],
                         strip[:, COL_PC0 + g:COL_PC0 + g + 1])
        p0 = pkpool.tile([P_B, F_C], BF16, tag="p0")
        nc.sync.dma_start(out=p0[:], in_=d4pk[b, g])
        delta_masked_sum(nc, pkpool, p0, asg_pk[g],
                         strip[:, COL_OBJ0 + g:COL_OBJ0 + g + 1])

    # ---------- finalize ----------
    ps_fin = psfpool.tile([1, NCOL], F32, tag="ps_fin")
    nc.tensor.matmul(ps_fin[:], lhsT=kones[:], rhs=strip[:], start=True, stop=True)
    fin = spool.tile([1, NCOL], F32, tag="fin")
    nc.vector.tensor_copy(fin[:], ps_fin[:])
    nc.sync.dma_start(out=out[b, :].unsqueeze(0), in_=fin[:])


def box_sl1(nc, pool, dm, m1, strip, j):
    nc.vector.tensor_tensor(dm[:], dm[:], m1[:], OP.mult)  # poison in place
    junk = pool.tile([P_B, F_B], BF16, tag="junk")
    nc.scalar.activation(junk[:], dm[:], AF.Relu, bias=-BETA, scale=1.0,
                         accum_out=strip[:, COL_RELUP + j:COL_RELUP + j + 1])
    junk2 = pool.tile([P_B, F_B], BF16, tag="junk")
    nc.scalar.activation(junk2[:], dm[:], AF.Relu, bias=-BETA, scale=-1.0,
                         accum_out=strip[:, COL_RELUN + j:COL_RELUN + j + 1])
    sqd = pool.tile([P_B, F_B], BF16, tag="sqd")
    nc.scalar.activation(sqd[:], dm[:], AF.Square)
    junk3 = pool.tile([P_B, F_B], BF16, tag="junk")
    nc.vector.tensor_scalar(junk3[:], sqd[:], BETA * BETA, 0.0, OP.min, OP.add,
                            accum_out=strip[:, COL_MSQ + j:COL_MSQ + j + 1])


def delta_masked_sum(nc, pool, pv, asg_pk, acc_col):
    """acc_col = sum_free (asg>=1) * [Delta(pv)/0.75];
    Delta(p) = -0.25*(1-p)^2*ln(p) + 0.75*p^2*ln(1-p); host multiplies 0.75."""
    lnp = pool.tile([P_B, F_C], BF16, tag="dc_lnp")
    nc.scalar.activation(lnp[:], pv[:], AF.Ln)
    ln1mp = pool.tile([P_B, F_C], BF16, tag="dc_ln1mp")
    nc.scalar.activation(ln1mp[:], pv[:], AF.Ln, bias=1.0, scale=-1.0)
    sqp = pool.tile([P_B, F_C], BF16, tag="dc_sqp")
    nc.scalar.activation(sqp[:], pv[:], AF.Square)
    sq1mp = pool.tile([P_B, F_C], BF16, tag="dc_sq1mp")
    nc.scalar.activation(sq1mp[:], pv[:], AF.Square, bias=1.0, scale=-1.0)
    nc.vector.tensor_tensor(sq1mp[:], sq1mp[:], lnp[:], OP.mult)   # (1-p)^2 ln p
    nc.vector.tensor_tensor(sqp[:], sqp[:], ln1mp[:], OP.mult)     # p^2 ln(1-p)
    dd = pool.tile([P_B, F_C], BF16, tag="dc_dd")
    nc.vector.scalar_tensor_tensor(dd[:], sq1mp[:], -1.0 / 3.0, sqp[:],
                                   OP.mult, OP.add)
    jk = pool.tile([P_B, F_C], BF16, tag="dc_jk")
    nc.vector.scalar_tensor_tensor(jk[:], asg_pk[:], 1.0, dd[:], OP.is_ge, OP.mult,
                                   accum_out=acc_col)


# ======================= host-side prep =======================

def make_consts():
    # sliding-window pack matrix (entries -1): cls tile tl uses
    # kdn[:, 240-6*tl : 366-6*tl], which has -1 at [p, 6*tl + p//21].
    kdn = np.zeros((126, 366), ml_dtypes.bfloat16)
    for p in range(126):
        kdn[p, 240 + p // NCH] = -1.0
    iota = (np.arange(126, dtype=np.float32) % NCH - 1.0).reshape(126, 1)
    ones_col = np.ones((P_B, 1), np.float32)
    return kdn, iota, ones_col


def make_pair_tables(gt):
    """[B, 5, 8, 8192] bf16: row r replicated 8x; entry (g0*64+g1) holds the
    (f[g0], f[g1]) bf16 pair for field r in [Gx, Gy, ln w, ln h, cls+6]."""
    gt = np.asarray(gt, np.float32)  # [B, G, 5]
    gx = gt[:, :, 0] + 0.5 * gt[:, :, 2]
    gy = gt[:, :, 1] + 0.5 * gt[:, :, 3]
    lnw = np.log(gt[:, :, 2])
    lnh = np.log(gt[:, :, 3])
    cl = gt[:, :, 4] + 6.0  # cls - 1 + 7
    f = np.stack([gx, gy, lnw, lnh, cl], axis=1)  # [B, 5, G]
    g0 = np.arange(4096) // 64
    g1 = np.arange(4096) % 64
    pair = np.stack([f[:, :, g0], f[:, :, g1]], axis=-1)  # [B, 5, 4096, 2]
    pair = pair.reshape(B, 5, 1, 8192).astype(ml_dtypes.bfloat16)
    return np.ascontiguousarray(np.broadcast_to(pair, (B, 5, 8, 8192)))


def make_pair_idx(assign):
    """[B, 128, IDXC] int16/uint16: pair index per anchor pair, wrapped
    16-partition round-robin per Q7 core (core c owns anchors [15000c,...))."""
    gidx = np.clip(np.asarray(assign, np.int64) - 1, 0, G - 1)
    pair = (gidx[:, 0::2] * 64 + gidx[:, 1::2])  # [B, A/2]
    if GATHER_KIND == "ic":
        pair = (pair * 2).astype(np.uint16)  # element offsets
    else:
        pair = pair.astype(np.int16)
    pair = pair.reshape(B, 8, NPAIR)
    pad = np.zeros((B, 8, NIDX - NPAIR), pair.dtype)
    pair = np.concatenate([pair, pad], axis=2)          # [B, 8, NIDX]
    pair = pair.reshape(B, 8, IDXC, 16)
    pair = np.swapaxes(pair, 2, 3)                      # [B, 8, 16, IDXC]
    return np.ascontiguousarray(pair.reshape(B, 128, IDXC))


def make_asg_packed(assign):
    """[B, NG, 120, 500] bf16 in the cls packed layout (row 6*tl+k)."""
    a = np.asarray(assign, np.float32).reshape(B, 6, NG, G_T, F_C)
    a = np.transpose(a, (0, 2, 3, 1, 4))  # [B, NG, tl, k, f]
    return np.ascontiguousarray(a.reshape(B, NG, P_B, F_C).astype(ml_dtypes.bfloat16))


def host_combine(parts):
    parts = parts.reshape(-1, NCOL).astype(np.float64)
    cls = (parts[:, COL_CLS0] + parts[:, COL_CLS0 + 1]
           + 0.75 * (parts[:, COL_PC0] + parts[:, COL_PC0 + 1]
                     + parts[:, COL_OBJ0] + parts[:, COL_OBJ0 + 1]))
    box = (parts[:, COL_RELUP:COL_RELUP + 4].sum(1)
           + parts[:, COL_RELUN:COL_RELUN + 4].sum(1)
           + (0.5 / BETA) * parts[:, COL_MSQ:COL_MSQ + 4].sum(1))
    npos = np.maximum(parts[:, COL_NPOS0] + parts[:, COL_NPOS0 + 1], 1.0)
    return np.float32(np.sum((cls + box) / npos) / B)


def make_in_maps(inputs):
    dt = np.asarray(inputs["dt"], np.float32)
    gt = np.asarray(inputs["gt"], np.float32)
    anchors = np.asarray(inputs["anchors"], np.float32)
    assign = np.asarray(inputs["assign"])
    dtb = dt.astype(ml_dtypes.bfloat16)
    dtbox = np.ascontiguousarray(dtb[:, 0:4, :])
    # cls rows tile-major: [B, s, (k,c), (i,f)]
    dtcls = np.ascontiguousarray(
        dtb[:, 4:, :].reshape(B, NCH, 6, NSLAB, 10, F_C)
        .transpose(0, 3, 2, 1, 4, 5).reshape(B, NSLAB, 126, 10 * F_C))
    d4 = np.ascontiguousarray(
        dtb[:, 4, :].reshape(B, 6, NG, G_T, F_C)
        .transpose(0, 2, 3, 1, 4).reshape(B, NG, P_B, F_C))
    ancT = np.ascontiguousarray(anchors.T)  # [4, A]
    asgb = assign.astype(ml_dtypes.bfloat16)
    asgpk = make_asg_packed(assign)
    idx = make_pair_idx(assign)
    tbl = make_pair_tables(gt)
    kdn, iota, ones_col = make_consts()
    in_maps = []
    for c in range(NCORES):
        sl = slice(c * BPC, (c + 1) * BPC)
        in_maps.append({
            "dtbox": np.ascontiguousarray(dtbox[sl]),
            "dtcls": np.ascontiguousarray(dtcls[sl]),
            "d4pk": np.ascontiguousarray(d4[sl]),
            "anc": ancT,
            "asgbx": np.ascontiguousarray(asgb[sl]),
            "asgpk": np.ascontiguousarray(asgpk[sl]),
            "idxD": np.ascontiguousarray(idx[sl]),
            "tblD": np.ascontiguousarray(tbl[sl]),
            "kdnD": kdn, "iotaD": iota, "onesD": ones_col,
        })
    return in_maps


_prog_cache = {}


def kernel(dt, gt, anchors, assign):
    from concourse.bass_utils import run_bass_kernel_spmd

    if "nc" not in _prog_cache:
        _prog_cache["nc"] = build_program()
    nc = _prog_cache["nc"]

    in_maps = make_in_maps({"dt": dt, "gt": gt, "anchors": anchors,
                            "assign": assign})
    results = run_bass_kernel_spmd(nc, in_maps, core_ids=list(range(NCORES))).results
    parts = np.stack([results[c]["out"] for c in range(NCORES)])
    return host_combine(parts)
